# revision 25
# baseline (speedup 1.0000x reference)
"""Expert-parallel MoE FFN kernel for Trainium2 (8 NeuronCores).

Strategy (sharding_hint: expert-parallel):
  - Host computes the gate in fp32 (softmax -> top-2 -> renormalize) and
    dispatches tokens to experts (the "all-to-all" happens in host staging,
    which is legal because kernel() receives FULL inputs).
  - Core e holds expert e's weights (bf16) and processes its gathered tokens
    (padded to a static capacity C) through the FFN:
        Y = (gelu(X @ W1 + b1) @ W2) * combine_scale
    All GEMMs run in bf16 on the PE with fp32 PSUM accumulation; gelu (exact,
    erf-based) is fused into the PSUM eviction on the scalar engine; the
    combine-weight scaling is fused into the second GEMM's PSUM eviction on
    the vector engine.
  - Host scatters per-expert outputs back (indices are unique per expert) and
    adds the (gate-weighted) b2 term.

Layouts (per core):
  xt  [H, C]  bf16   gathered tokens, transposed (contraction dim on partitions)
  w1  [H, FF] bf16   natural layout == lhsT for GEMM1
  w2  [FF, H] bf16   natural layout == lhsT for GEMM2 (stationary)
  b1p [128, FF/128]  f32, column ff = b1[ff*128:(ff+1)*128]
  y   [H, C]  f32    transposed per-slot FFN output (unscaled)

GEMM1 produces Hmid^T (FF on partitions); GEMM2 keeps tokens on the moving
operand (cycles scale with the exact token count, not 128-padded tiles) and
produces Y^T. The combine-weight scale and the final transpose happen on the
host during the scatter — zero transposes or gather/scatter on device.
"""

import sys

if "/opt/trn_rl_repo" not in sys.path:
    sys.path.insert(0, "/opt/trn_rl_repo")

import numpy as np
import ml_dtypes

H = 1024          # hidden size
E = 8             # experts == cores
TOPK = 2
FF = 4 * H        # expert hidden dim
P = 128           # SBUF partitions
NB = 384          # token block (matmul free dim, <= 512 PSUM bank)
NH = 512          # GEMM2 output free-dim tile

_prog_cache: dict[int, object] = {}
LAST_RESULTS = None  # BassKernelResults of the most recent run (for test harness)
TRACE = False        # test harness can set kernel.TRACE = True for profiling
ACT_OVERRIDE = None  # sim-only: CoreSim lacks Gelu; tests may set e.g. "Relu"
LAST_CALL = None     # (nc, in_maps) of the most recent run, for re-runs


def _seg_blocks(A: int, first: int = 0, last: int = 0):
    """Split A token columns into near-equal blocks, each <= 512 (PSUM bank)
    and large enough (>= ~240) that LDWEIGHTS stays hidden under matmuls.

    first/last: carve a fixed-size block off the front/back (0 = no carve).
    A small first GEMM1 block shrinks the startup x-DMA critical path; a
    small last GEMM2 block shrinks the output-DMA drain tail.
    """
    blocks = []
    t = 0
    if first and A >= first + 240:
        blocks.append((0, first))
        t = first
    end = A
    carve_last = bool(last) and (end - t) >= last + 240
    if carve_last:
        end = A - last
    mid = end - t
    nblk = max(1, -(-mid // 512))
    base = mid // nblk
    rem = mid % nblk
    for i in range(nblk):
        nb = base + (1 if i < rem else 0)
        blocks.append((t, nb))
        t += nb
    if carve_last:
        blocks.append((end, last))
    return blocks


def _build_program(segs: tuple[int, ...]):
    """Build + compile the per-core SPMD Bass program.

    segs: token-slot capacity per segment. Each segment processes one
    expert-shard with its own weight set; weight SBUF slots are reused
    across segments (Tile's WAR deps overlap the next segment's weight
    DMA with the previous segment's compute).

    DRAM I/O (S = len(segs), Ctot = sum(segs)):
      xt  [H, Ctot]  bf16, w1 [S*H, FF] bf16, w2 [S*FF, H] bf16,
      b1p [P, S*KF] f32, y [H, Ctot] f32 (unscaled YT)
    """
    from contextlib import ExitStack

    from concourse import bacc
    import concourse.mybir as mybir
    import concourse.tile as tile

    dt = mybir.dt
    KH = H // P            # 8  contraction chunks for GEMM1
    KF = FF // P           # 32 contraction chunks for GEMM2
    S = len(segs)
    Ctot = sum(segs)
    g1_blocks = [_seg_blocks(A, first=(256 if si == 0 else 0))
                 for si, A in enumerate(segs)]
    g2_blocks = [_seg_blocks(A, last=(256 if si == S - 1 else 0))
                 for si, A in enumerate(segs)]
    NBMAX = max(nb for bl in (g1_blocks + g2_blocks) for _, nb in bl)

    nc = bacc.Bacc(None, target_bir_lowering=False, debug=False)

    xt = nc.dram_tensor("xt", [H, Ctot], dt.bfloat16, kind="ExternalInput")
    w1 = nc.dram_tensor("w1", [S * H, FF], dt.bfloat16, kind="ExternalInput")
    w2 = nc.dram_tensor("w2", [S * FF, H], dt.bfloat16, kind="ExternalInput")
    b1p = nc.dram_tensor("b1p", [P, S * KF], dt.float32, kind="ExternalInput")
    y = nc.dram_tensor("y", [H, Ctot], dt.bfloat16, kind="ExternalOutput")

    with ExitStack() as ctx:
        tc = ctx.enter_context(tile.TileContext(nc))
        wpool = ctx.enter_context(tc.tile_pool(name="wpool", bufs=1))
        xpool = ctx.enter_context(tc.tile_pool(name="xpool", bufs=2))
        hpool = ctx.enter_context(tc.tile_pool(name="hpool", bufs=1))
        psA = ctx.enter_context(tc.tile_pool(name="psA", bufs=3, space="PSUM"))
        psB = ctx.enter_context(tc.tile_pool(name="psB", bufs=3, space="PSUM"))
        opool = ctx.enter_context(tc.tile_pool(name="opool", bufs=4))

        act = getattr(mybir.ActivationFunctionType, ACT_OVERRIDE or "Gelu")
        xt_r = xt[:, :].rearrange("(k p) t -> p k t", p=P)
        CSMAX = max(segs)

        c0 = 0
        for si, A in enumerate(segs):
            blocks = g1_blocks[si]
            w1_r = w1[si * H:(si + 1) * H, :].rearrange("(k p) f -> p k f", p=P)
            w2_r = w2[si * FF:(si + 1) * FF, :].rearrange("(k p) h -> p k h", p=P)

            # --- segment inputs --------------------------------------------
            # Few large multi-chunk DMAs (descriptor issue on sync is the
            # startup bottleneck; one big DMA runs at full fabric BW), in
            # consumption order: block-0 tokens, bias, W1 pieces sized so
            # the first matmul group's critical prefix is ~1MB, W2, rest.
            xtile = xpool.tile([P, KH, CSMAX], dt.bfloat16, tag="xtile",
                               name="xtile")
            w1t = wpool.tile([P, KH, FF], dt.bfloat16, tag="w1t", name="w1t")
            w2t = wpool.tile([P, KF, H], dt.bfloat16, tag="w2t", name="w2t")
            b1t = xpool.tile([P, KF], dt.float32, tag="b1t", name="b1t")

            nb0 = blocks[0][1]
            nc.sync.dma_start(out=xtile[:, :, 0:nb0],
                              in_=xt_r[:, :, c0:c0 + nb0])
            nc.sync.dma_start(out=b1t[:], in_=b1p[:, si * KF:(si + 1) * KF])
            # Fine-grained early W1 edges: block 0's groups consume one
            # 128-col chunk per ~0.85us, so supply must not fall behind
            # while the startup DMA ramp is still contended. The x
            # remainder is only needed for block 1 (~27us in), so it
            # queues after all of W1 rather than in the middle of it.
            w1_edges = [0, P, 4 * P, FF // 4, FF // 2, 3 * FF // 4, FF]
            for fb in range(len(w1_edges) - 1):
                nc.sync.dma_start(
                    out=w1t[:, :, w1_edges[fb]:w1_edges[fb + 1]],
                    in_=w1_r[:, :, w1_edges[fb]:w1_edges[fb + 1]],
                )
            if A > nb0:
                nc.sync.dma_start(out=xtile[:, :, nb0:A],
                                  in_=xt_r[:, :, c0 + nb0:c0 + A])
            nc.sync.dma_start(out=w2t[:, :, :], in_=w2_r[:, :, :])

            # --- compute: all GEMM1 blocks, then all GEMM2 blocks ----------
            # GEMM2 must not start before ~1/2 of the segment's compute has
            # elapsed or the W2 DMA (8.4MB) is still in flight (HBM-bound).
            hblk = hpool.tile([P, KF, CSMAX], dt.bfloat16, tag="hblk",
                              name="hblk")
            for t0, nb in blocks:
                # GEMM1: HmidT[f, t] = gelu(sum_h W1[h, f]*xt[h, t] + b1[f])
                for ff in range(KF):
                    pa = psA.tile([P, NBMAX], dt.float32, tag="pa", name="pa")
                    warm_n = 0
                    if si == 0 and t0 == 0 and ff == 0:
                        # HAM pre-warm: the PE would idle ~7us waiting for
                        # the first input DMAs and then run its first
                        # ~3.4us of matmuls at the cold 1.2GHz clock.
                        # Accumulate zero-matmuls (numeric no-op) into this
                        # first group's PSUM while waiting — same psum dep
                        # chain, so they are forced to the stream head.
                        warm_n = 42
                        warm = wpool.tile([P, NBMAX], dt.bfloat16,
                                          tag="warm", name="warm")
                        nc.vector.memset(warm[:, :nb], 0.0)
                        for i in range(warm_n):
                            nc.tensor.matmul(
                                pa[:, :nb],
                                lhsT=warm[:, :P],
                                rhs=warm[:, :nb],
                                start=(i == 0),
                                stop=False,
                            )
                    for k in range(KH):
                        nc.tensor.matmul(
                            pa[:, :nb],
                            lhsT=w1t[:, k, ff * P:(ff + 1) * P],
                            rhs=xtile[:, k, t0:t0 + nb],
                            start=(k == 0 and warm_n == 0),
                            stop=(k == KH - 1),
                        )
                    nc.scalar.activation(
                        hblk[:, ff, t0:t0 + nb],
                        pa[:, :nb],
                        act,
                        bias=b1t[:, ff:ff + 1],
                    )
            for t0, nb in g2_blocks[si]:
                # GEMM2: YT[h, t] = sum_f W2[f, h] * HmidT[f, t]
                # W2 chunks stationary; tokens stay on the moving side so
                # cycles scale with the exact token count.
                for ht in range(H // P):
                    pb = psB.tile([P, NBMAX], dt.float32, tag="pb", name="pb")
                    for k in range(KF):
                        nc.tensor.matmul(
                            pb[:, :nb],
                            lhsT=w2t[:, k, ht * P:(ht + 1) * P],
                            rhs=hblk[:, k, t0:t0 + nb],
                            start=(k == 0),
                            stop=(k == KF - 1),
                        )
                    ot = opool.tile([P, NBMAX], dt.bfloat16, tag="ot", name="ot")
                    nc.vector.tensor_copy(ot[:, :nb], pb[:, :nb])
                    nc.sync.dma_start(
                        out=y[ht * P:(ht + 1) * P, c0 + t0:c0 + t0 + nb],
                        in_=ot[:, :nb],
                    )
            c0 += A

    nc.compile()
    return nc


def _get_program(segs: tuple[int, ...]):
    if segs not in _prog_cache:
        _prog_cache[segs] = _build_program(segs)
    return _prog_cache[segs]


def _route(xf: np.ndarray, Wg: np.ndarray, bg: np.ndarray):
    """fp32 gate: softmax -> top-2 (stable order, matches jax top_k) -> renorm."""
    logits = xf @ np.asarray(Wg, np.float32) + np.asarray(bg, np.float32)
    m = logits.max(axis=1, keepdims=True)
    p = np.exp(logits - m, dtype=np.float32)
    p /= p.sum(axis=1, keepdims=True)
    order = np.argsort(-p, axis=1, kind="stable")
    idx = order[:, :TOPK]
    pv = np.take_along_axis(p, idx, axis=1)
    vals = (pv / pv.sum(axis=1, keepdims=True)).astype(np.float32)
    return idx, vals


def kernel(x, Wg, bg, W1, b1, W2, b2):
    global LAST_RESULTS
    from concourse.bass_utils import run_bass_kernel_spmd

    x = np.asarray(x, np.float32)
    xf = x.reshape(-1, H)
    T = xf.shape[0]

    idx, vals = _route(xf, Wg, bg)

    counts = np.bincount(idx.ravel(), minlength=E)
    # Half-expert load balancing: each expert's tokens split into two
    # halves on two different cores; the 4 lightest experts fill every
    # core's segment 1, the 4 heaviest fill segment 2. Per-core capacity
    # is then max-minor/2 + max-major/2 instead of the single max count.
    # Minors go FIRST: their single wide block consumes W1 slowly enough
    # to be fed during the DMA ramp-up at kernel start.
    eorder = np.argsort(-counts, kind="stable")
    majors, minors = eorder[E // 2:], eorder[:E // 2]
    S1 = max(1, int(-(-counts[majors].max() // 2)))
    S2 = max(1, int(-(-counts[minors].max() // 2)))
    segs = (S1, S2)

    nc = _get_program(segs)

    bf16 = ml_dtypes.bfloat16
    W1 = np.asarray(W1, np.float32)
    W2 = np.asarray(W2, np.float32)
    b1 = np.asarray(b1, np.float32)
    KF = FF // P

    shards = {}
    for e in range(E):
        sel = idx == e                      # [T, 2]; at most one True per row
        ids = np.nonzero(sel.any(axis=1))[0]
        shards[e] = (ids, vals[sel])        # row-major => aligned with ids

    in_maps = []
    ids_list = []
    for pr in range(E // 2):
        ea, eb = int(majors[pr]), int(minors[E // 2 - 1 - pr])
        ids_a, sc_a = shards[ea]
        ids_b, sc_b = shards[eb]
        ha, hb = -(-ids_a.size // 2), -(-ids_b.size // 2)
        w1cat = np.concatenate([W1[ea], W1[eb]], axis=0).astype(bf16)
        w2cat = np.concatenate([W2[ea], W2[eb]], axis=0).astype(bf16)
        b1cat = np.ascontiguousarray(np.concatenate(
            [b1[ea].reshape(KF, P).T, b1[eb].reshape(KF, P).T], axis=1
        ))
        for half in range(2):
            pa = slice(0, ha) if half == 0 else slice(ha, ids_a.size)
            pb = slice(0, hb) if half == 0 else slice(hb, ids_b.size)
            na, nb_ = ids_a[pa].size, ids_b[pb].size
            xe = np.zeros((S1 + S2, H), np.float32)
            xe[:na] = xf[ids_a[pa]]
            xe[S1:S1 + nb_] = xf[ids_b[pb]]
            in_maps.append({
                "xt": np.ascontiguousarray(xe.T).astype(bf16),
                "w1": w1cat,
                "w2": w2cat,
                "b1p": b1cat,
            })
            ids_list.append(
                ((ids_a[pa], sc_a[pa]), (ids_b[pb], sc_b[pb]))
            )

    global LAST_CALL
    LAST_CALL = (nc, in_maps)
    LAST_RESULTS = run_bass_kernel_spmd(nc, in_maps, list(range(E)), trace=TRACE)

    out = np.zeros((T, H), np.float32)
    for c in range(E):
        (ids_a, sc_a), (ids_b, sc_b) = ids_list[c]
        yt = np.asarray(LAST_RESULTS.results[c]["y"], np.float32)  # [H, S1+S2]
        if ids_a.size:
            out[ids_a] += yt[:, :ids_a.size].T * sc_a[:, None]
        if ids_b.size:
            out[ids_b] += yt[:, S1:S1 + ids_b.size].T * sc_b[:, None]

    b2 = np.asarray(b2, np.float32)
    out += vals[:, 0:1] * b2[idx[:, 0]] + vals[:, 1:2] * b2[idx[:, 1]]
    return out.reshape(x.shape)

